# revision 32
# baseline (speedup 1.0000x reference)
"""Trainium2 Bass kernel for per-position head-mixing attention.

Math (per position p): Qh,Kh,Vh = reshape(q/k/v[p], [16, 64]);
L = Qh @ Kh.T / 8; W = softmax(L, axis=-1); out[p] = W @ Vh.

Strategy:
  * Pure data parallel over 8 cores (4096 positions each).
  * The kernel is HBM-bandwidth bound.  Measured per-SDMA-engine read
    rates under full 8-core load favor 128-partition ops with ~4 KiB
    per-partition rows over a fully sequential DRAM region (~18-19.5
    GB/s/engine vs ~16.2 for 64-partition ops), so EVERY load is shaped
    that way:
      - 16 chunks of 256 positions.  qk DRAM slab [c, side, 128, 2048]:
        partition rows 0-63 hold the d-dim of groups 0-15 ("rail A"),
        rows 64-127 hold groups 16-31 ("rail B").  One [128, 2048] op per
        side per chunk.  v is [128, 2080] (4160 B rows, contiguous).
      - chunk 0 is issued at the head of both HWDGE rings (sync + scalar)
        and the mask constant at the head of the gpsimd ring, so the ops
        gating the first compute never queue behind bulk prefetch.
      - 4-deep chunk prefetch (deeper buffering just moves work into a
        post-load compute tail), flat software pipeline over all 64
        batches, half-chunk output stores spread in time.
  * Host pre-transposes q,k to [d, ...] layout (exact, host-side) and casts
    to bf16 so every device DMA is large and contiguous.
  * Per group of 8 positions, one 64x128x128 matmul (rail A reads
    partitions 0-63, rail B partitions 64-127; bass derives tile_position
    (0,0)/(64,0)) computes all 16x16 logit blocks of the group.  Each
    batch of 8 groups ALTERNATES rails (A,B,A,B,...) so consecutive
    matmuls target disjoint PE row-halves and run concurrently
    (per-subarray packing, ~2x matmul-1 throughput).
  * exp on ScalarE (scale=1/8 fused), batched over 8 groups; VectorE then
    multiplies by a 0/1 block-diagonal mask (exact zeros for the
    cross-position entries).
  * Second matmul W' @ [V | 1] gives numerator and softmax denominator in
    one pass; VectorE copies both to SBUF as fp16 and the HOST performs
    the final numerator/denominator division (exact reorganization, off
    the device's critical path).
"""

import sys

if "/opt/trn_rl_repo" not in sys.path:
    sys.path.insert(0, "/opt/trn_rl_repo")

from contextlib import ExitStack

import ml_dtypes
import numpy as np

import concourse.bass as bass
from concourse import bacc, mybir, tile
from concourse.bass_utils import run_bass_kernel_spmd

BF16 = mybir.dt.bfloat16
F16 = mybir.dt.float16
F32 = mybir.dt.float32
NPBF16 = ml_dtypes.bfloat16

N_CORES = 8
S_TOT = 4 * 8192          # flattened (batch, seq) positions
H, D = 16, 64             # heads, key size
N_PC = S_TOT // N_CORES   # positions per core
CH = 256                  # positions per chunk
GC = CH // 8              # 8-position groups per chunk (32)
NT = N_PC // CH           # chunks per core (16)
B = 8                     # groups per psum/exp batch
NBC = GC // B             # batches per chunk (4)
NQK = 10                  # qk chunk buffers
PF = 5                    # chunk prefetch depth
SCALE = 0.125             # 1/sqrt(64)
QW = GC // 2 * 128        # columns per side in a qk buffer (2048)


def _slot_group(bl: int, j: int) -> int:
    """Group handled by pipeline slot j of batch bl (rails interleaved)."""
    return 16 * (j % 2) + bl * 4 + j // 2


def _p1_col(j: int) -> int:
    """p1/w column block for slot j.  Consecutive slots run concurrently in
    the PE (disjoint row-halves), so route them to different PSUM banks:
    even slots -> bank 0 columns, odd slots -> bank 1 columns."""
    return (j % 2) * 512 + (j // 2) * 128

_CACHE = {}


def _build_program(nt: int, n_cores: int):
    nc = bacc.Bacc(
        "TRN2", target_bir_lowering=False, debug=False, num_devices=n_cores
    )
    # [chunk, {q,k}, rail-packed rows, G/2, H, 8]
    qk = nc.dram_tensor(
        "qk", [nt, 2, 128, GC // 2, H, 8], BF16, kind="ExternalInput"
    ).ap()
    mk = nc.dram_tensor("mk", [128, B * 128], BF16, kind="ExternalInput").ap()
    vr = nc.dram_tensor("vr", [nt, H, 8, GC, 65], BF16, kind="ExternalInput").ap()
    # numerator (64) + softmax denominator (1) per (position, head); the
    # host performs the final division
    out = nc.dram_tensor("o", [nt, H, 8, GC, 65], F16, kind="ExternalOutput").ap()

    NB_TOT = nt * NBC          # total batches (64)

    with tile.TileContext(nc) as tc, ExitStack() as ctx:
        qk_pool = ctx.enter_context(tc.tile_pool(name="qk", bufs=1))
        m_pool = ctx.enter_context(tc.tile_pool(name="m", bufs=1))
        v_pool = ctx.enter_context(tc.tile_pool(name="v", bufs=NQK))
        o_pool = ctx.enter_context(tc.tile_pool(name="o", bufs=4))
        w_pool = ctx.enter_context(tc.tile_pool(name="w", bufs=4))
        wm_pool = ctx.enter_context(tc.tile_pool(name="wm", bufs=4))
        p1_pool = ctx.enter_context(tc.tile_pool(name="p1", bufs=2, space="PSUM"))
        p2_pool = ctx.enter_context(tc.tile_pool(name="p2", bufs=2, space="PSUM"))

        qk_bufs = [
            qk_pool.tile([128, 2 * QW], BF16, tag=f"qk{j}", name=f"qkbuf{j}")
            for j in range(NQK)
        ]
        mask_t = m_pool.tile([128, B * 128], BF16, tag="mask", name="mask")

        def load_qk(c, eng, sides=(0, 1)):
            buf = qk_bufs[c % NQK]
            for side in sides:
                src = qk[c, side].rearrange("p a b c -> p (a b c)")  # [128, 2048]
                eng.dma_start(buf[:, side * QW : (side + 1) * QW], src)

        def load_v(c, eng):
            v_t = v_pool.tile([128, GC * 65], BF16)
            src_v = vr[c].rearrange("k p g e -> (k p) (g e)")
            eng.dma_start(v_t[:], src_v)
            return v_t

        # Initial prefetch.  Mask constant first on gpsimd (gates the first
        # DVE op); chunk 0's two qk ops at the heads of the two HWDGE
        # rings; v0 right behind the mask.  Chunks 1-3 queue behind.
        v_tiles = {}
        nc.gpsimd.dma_start(mask_t[:], mk)
        load_qk(0, eng=nc.sync, sides=(0,))
        load_qk(0, eng=nc.scalar, sides=(1,))
        v_tiles[0] = load_v(0, eng=nc.gpsimd)
        load_qk(1, eng=nc.sync, sides=(0,))
        load_qk(1, eng=nc.scalar, sides=(1,))
        v_tiles[1] = load_v(1, eng=nc.gpsimd)
        load_qk(2, eng=nc.sync, sides=(0,))
        load_qk(2, eng=nc.scalar, sides=(1,))
        v_tiles[2] = load_v(2, eng=nc.gpsimd)
        load_qk(3, eng=nc.sync, sides=(0,))
        load_qk(3, eng=nc.scalar, sides=(1,))
        v_tiles[3] = load_v(3, eng=nc.gpsimd)
        for c0 in range(4, PF):
            load_qk(c0, eng=nc.gpsimd)
            v_tiles[c0] = load_v(c0, eng=nc.gpsimd)

        o_tiles = {}
        pending = []
        for bb in range(NB_TOT + 2):
            # Stage order within an iteration is chosen so each engine's
            # FIFO sees its ready-soonest work first:
            #   PE:  MM1(bb) then MM2(bb-2)      (rail pairs stay adjacent)
            #   DVE: cast(bb-2) then mask(bb)    (cast's input is already
            #        computed; mask waits on the exp still in flight)
            w = None
            if bb < NB_TOT:
                c, bl = bb // NBC, bb % NBC
                if bl == 0:
                    if c + PF < nt:
                        load_qk(c + PF, eng=nc.gpsimd)
                        v_tiles[c + PF] = load_v(c + PF, eng=nc.gpsimd)
                    o_tiles[c] = o_pool.tile([128, GC * 65], F16, name="o_t")
                qk_t = qk_bufs[c % NQK]
                p1 = p1_pool.tile([128, B * 128], F32)
                for j in range(B):
                    g = _slot_group(bl, j)
                    rail, gl = g // 16, g % 16
                    rs = slice(rail * 64, rail * 64 + 64)
                    pc = _p1_col(j)
                    nc.tensor.matmul(
                        p1[:, pc : pc + 128],
                        lhsT=qk_t[rs, QW + gl * 128 : QW + (gl + 1) * 128],
                        rhs=qk_t[rs, gl * 128 : (gl + 1) * 128],
                        start=True,
                        stop=True,
                    )
                w = w_pool.tile([128, B * 128], BF16)
                nc.scalar.activation(
                    w[:], p1[:], mybir.ActivationFunctionType.Exp, scale=SCALE
                )
            if bb >= 2:
                wp, bp = pending.pop(0)
                cp, blp = bp // NBC, bp % NBC
                v_t = v_tiles[cp]
                o_t = o_tiles[cp]
                # slots 0-3 write PSUM bank 0 (cols 0..259), slots 4-7 bank 1
                # (cols 512..771): every matmul-2 output stays inside one
                # bank, and ONE strided CAST copies both halves out
                p2 = p2_pool.tile([128, B * 128], F32)
                for j in range(B):
                    g = _slot_group(blp, j)
                    pc = _p1_col(j)
                    oc = (j // 4) * 512 + (j % 4) * 65
                    nc.tensor.matmul(
                        p2[:, oc : oc + 65],
                        lhsT=wp[:, pc : pc + 128],
                        rhs=v_t[:, g * 65 : (g + 1) * 65],
                        start=True,
                        stop=True,
                    )
                ob = blp * B * 65
                p2v = p2[:].rearrange("p (h c) -> p h c", c=512)
                ovv = o_t[:, ob : ob + 520].rearrange("p (h c) -> p h c", c=260)
                nc.vector.tensor_copy(ovv, p2v[:, :, 0:260])
                if blp % 2 == 1 or cp == nt - 1:
                    # store finished half-chunks to spread write traffic; the
                    # last chunk stores quarter-chunks to shorten the drain
                    oflat = out[cp].rearrange("k p g e -> (k p) (g e)")
                    qw_ = GC * 65 // 4
                    if cp == nt - 1:
                        hsel = slice(blp * qw_, (blp + 1) * qw_)
                    else:
                        hsel = slice(0, 2 * qw_) if blp == 1 else slice(
                            2 * qw_, 4 * qw_
                        )
                    nc.sync.dma_start(oflat[:, hsel], o_t[:, hsel])
                    if blp == NBC - 1:
                        del v_tiles[cp], o_tiles[cp]
            if w is not None:
                wm = wm_pool.tile([128, B * 128], BF16)
                nc.vector.tensor_tensor(
                    wm[:], w[:], mask_t[:], op=mybir.AluOpType.mult
                )
                pending.append((wm, bb))

    nc.compile()
    return nc


def _prep_qk(qslab: np.ndarray, kslab: np.ndarray, nt: int) -> np.ndarray:
    """Two [nt*CH, 1024] fp32 slabs -> [nt, 2, 128, GC//2, H, 8] bf16.

    Per (chunk, side): one contiguous [128, 2048] block, 4 KiB rows, read
    fully sequentially by one DMA.  Rows 0-63 = d-dim of groups 0-15,
    rows 64-127 = d-dim of groups 16-31."""
    full = np.empty((nt, 2, 2, 64, GC // 2, H, 8), dtype=NPBF16)
    for s, slab in enumerate((qslab, kslab)):
        a = slab.reshape(nt, 2, GC // 2, 8, H, D)   # [c, rail, g, p, h, d]
        full[:, s] = a.transpose(0, 1, 5, 2, 4, 3).astype(NPBF16)
    return full.reshape(nt, 2, 128, GC // 2, H, 8)


def _mask_const() -> np.ndarray:
    """[128, B*128] bf16 0/1 block-diagonal mask.

    Row i = (h', p') of the k-side, column j = (h, p) of the q-side within
    a group; entry is 1 iff p == p' (heads attend only within their own
    position), tiled across the B=8 groups of a batch."""
    m = np.zeros((128, 128), dtype=NPBF16)
    for p in range(8):
        m[(np.arange(H) * 8 + p)[:, None], (np.arange(H) * 8 + p)[None, :]] = 1
    return np.tile(m, (1, B))


def _prep_v(slab: np.ndarray, nt: int) -> np.ndarray:
    """[nt*CH, 1024] fp32 -> [nt, H, 8, GC, 65] bf16 with ones column."""
    a = slab.reshape(nt, GC, 8, H, D)
    full = np.empty((nt, H, 8, GC, 65), dtype=NPBF16)
    full[..., :64] = a.transpose(0, 3, 2, 1, 4).astype(NPBF16)
    full[..., 64] = NPBF16(1.0)
    return full


def kernel(q: np.ndarray, k: np.ndarray, v: np.ndarray) -> np.ndarray:
    bshape = q.shape
    qf = np.ascontiguousarray(np.asarray(q, dtype=np.float32)).reshape(S_TOT, H * D)
    kf = np.ascontiguousarray(np.asarray(k, dtype=np.float32)).reshape(S_TOT, H * D)
    vf = np.ascontiguousarray(np.asarray(v, dtype=np.float32)).reshape(S_TOT, H * D)

    key = (NT, N_CORES)
    if key not in _CACHE:
        _CACHE[key] = _build_program(*key)
    nc = _CACHE[key]

    mk = _mask_const()
    in_maps = []
    for c in range(N_CORES):
        s0, s1 = c * N_PC, (c + 1) * N_PC
        in_maps.append(
            {
                "qk": _prep_qk(qf[s0:s1], kf[s0:s1], NT),
                "mk": mk,
                "vr": _prep_v(vf[s0:s1], NT),
            }
        )

    res = run_bass_kernel_spmd(nc, in_maps, core_ids=list(range(N_CORES)))

    # device column blocks are in pipeline-slot order; map slot -> group
    perm = np.empty(GC, dtype=np.int64)
    for bl in range(NBC):
        for j in range(B):
            perm[_slot_group(bl, j)] = bl * B + j

    out = np.empty((S_TOT, H * D), dtype=np.float32)
    for c in range(N_CORES):
        o = res.results[c]["o"].astype(np.float32)  # [NT, H, 8, GC(slots), 65]
        o = o[:, :, :, perm, :]                     # -> group order
        o = o[..., :64] / o[..., 64:65]             # softmax normalization
        out[c * N_PC : (c + 1) * N_PC] = (
            o.transpose(0, 3, 2, 1, 4).reshape(N_PC, H * D)
        )
    return out.reshape(bshape)


# revision 33
# speedup vs baseline: 1.0963x; 1.0963x over previous
"""Trainium2 Bass kernel for per-position head-mixing attention.

Math (per position p): Qh,Kh,Vh = reshape(q/k/v[p], [16, 64]);
L = Qh @ Kh.T / 8; W = softmax(L, axis=-1); out[p] = W @ Vh.

Strategy:
  * Pure data parallel over 8 cores (4096 positions each).
  * The kernel is HBM-bandwidth bound.  Measured per-SDMA-engine read
    rates under full 8-core load favor 128-partition ops with ~4 KiB
    per-partition rows over a fully sequential DRAM region (~18-19.5
    GB/s/engine vs ~16.2 for 64-partition ops), so EVERY load is shaped
    that way:
      - 16 chunks of 256 positions.  qk DRAM slab [c, side, 128, 2048]:
        partition rows 0-63 hold the d-dim of groups 0-15 ("rail A"),
        rows 64-127 hold groups 16-31 ("rail B").  One [128, 2048] op per
        side per chunk.  v is [128, 2080] (4160 B rows, contiguous).
      - chunk 0 is issued at the head of both HWDGE rings (sync + scalar)
        and the mask constant at the head of the gpsimd ring, so the ops
        gating the first compute never queue behind bulk prefetch.
      - 4-deep chunk prefetch (deeper buffering just moves work into a
        post-load compute tail), flat software pipeline over all 64
        batches, half-chunk output stores spread in time.
  * Host pre-transposes q,k to [d, ...] layout (exact, host-side) and casts
    to bf16 so every device DMA is large and contiguous.
  * Per group of 8 positions, one 64x128x128 matmul (rail A reads
    partitions 0-63, rail B partitions 64-127; bass derives tile_position
    (0,0)/(64,0)) computes all 16x16 logit blocks of the group.  Each
    batch of 8 groups ALTERNATES rails (A,B,A,B,...) so consecutive
    matmuls target disjoint PE row-halves and run concurrently
    (per-subarray packing, ~2x matmul-1 throughput).
  * exp on ScalarE (scale=1/8 fused), batched over 8 groups; VectorE then
    multiplies by a 0/1 block-diagonal mask (exact zeros for the
    cross-position entries).
  * Second matmul W' @ [V | 1] gives numerator and softmax denominator in
    one pass; VectorE copies both to SBUF as fp16 and the HOST performs
    the final numerator/denominator division (exact reorganization, off
    the device's critical path).
"""

import sys

if "/opt/trn_rl_repo" not in sys.path:
    sys.path.insert(0, "/opt/trn_rl_repo")

from contextlib import ExitStack

import ml_dtypes
import numpy as np

import concourse.bass as bass
from concourse import bacc, mybir, tile
from concourse.bass_utils import run_bass_kernel_spmd

BF16 = mybir.dt.bfloat16
F16 = mybir.dt.float16
F32 = mybir.dt.float32
NPBF16 = ml_dtypes.bfloat16

N_CORES = 8
S_TOT = 4 * 8192          # flattened (batch, seq) positions
H, D = 16, 64             # heads, key size
N_PC = S_TOT // N_CORES   # positions per core
CH = 256                  # positions per chunk
GC = CH // 8              # 8-position groups per chunk (32)
NT = N_PC // CH           # chunks per core (16)
B = 8                     # groups per psum/exp batch
NBC = GC // B             # batches per chunk (4)
NQK = 10                  # qk chunk buffers
PF = 5                    # chunk prefetch depth
SCALE = 0.125             # 1/sqrt(64)
QW = GC // 2 * 128        # columns per side in a qk buffer (2048)


def _slot_group(bl: int, j: int) -> int:
    """Group handled by pipeline slot j of batch bl (rails interleaved)."""
    return 16 * (j % 2) + bl * 4 + j // 2


def _p1_col(j: int) -> int:
    """p1/w column block for slot j.  Consecutive slots run concurrently in
    the PE (disjoint row-halves), so route them to different PSUM banks:
    even slots -> bank 0 columns, odd slots -> bank 1 columns."""
    return (j % 2) * 512 + (j // 2) * 128

_CACHE = {}


def _build_program(nt: int, n_cores: int):
    nc = bacc.Bacc(
        "TRN2", target_bir_lowering=False, debug=False, num_devices=n_cores
    )
    # [chunk, {q,k}, rail-packed rows, G/2, H, 8]
    qk = nc.dram_tensor(
        "qk", [nt, 2, 128, GC // 2, H, 8], BF16, kind="ExternalInput"
    ).ap()
    mk = nc.dram_tensor("mk", [128, B * 128], BF16, kind="ExternalInput").ap()
    vr = nc.dram_tensor("vr", [nt, H, 8, GC, 65], BF16, kind="ExternalInput").ap()
    # numerator (64) + softmax denominator (1) per (position, head); the
    # host performs the final division
    out = nc.dram_tensor("o", [nt, H, 8, GC, 65], F16, kind="ExternalOutput").ap()

    NB_TOT = nt * NBC          # total batches (64)

    with tile.TileContext(nc) as tc, ExitStack() as ctx:
        qk_pool = ctx.enter_context(tc.tile_pool(name="qk", bufs=1))
        m_pool = ctx.enter_context(tc.tile_pool(name="m", bufs=1))
        v_pool = ctx.enter_context(tc.tile_pool(name="v", bufs=NQK))
        o_pool = ctx.enter_context(tc.tile_pool(name="o", bufs=4))
        w_pool = ctx.enter_context(tc.tile_pool(name="w", bufs=4))
        wm_pool = ctx.enter_context(tc.tile_pool(name="wm", bufs=4))
        p1_pool = ctx.enter_context(tc.tile_pool(name="p1", bufs=2, space="PSUM"))
        p2_pool = ctx.enter_context(tc.tile_pool(name="p2", bufs=2, space="PSUM"))

        qk_bufs = [
            qk_pool.tile([128, 2 * QW], BF16, tag=f"qk{j}", name=f"qkbuf{j}")
            for j in range(NQK)
        ]
        mask_t = m_pool.tile([128, B * 128], BF16, tag="mask", name="mask")

        def load_qk(c, eng, sides=(0, 1)):
            buf = qk_bufs[c % NQK]
            for side in sides:
                src = qk[c, side].rearrange("p a b c -> p (a b c)")  # [128, 2048]
                eng.dma_start(buf[:, side * QW : (side + 1) * QW], src)

        def load_v(c, eng):
            v_t = v_pool.tile([128, GC * 65], BF16)
            src_v = vr[c].rearrange("k p g e -> (k p) (g e)")
            eng.dma_start(v_t[:], src_v)
            return v_t

        # Initial prefetch.  Mask constant first on gpsimd (gates the first
        # DVE op); chunk 0's two qk ops at the heads of the two HWDGE
        # rings; v0 right behind the mask.  Chunks 1-3 queue behind.
        v_tiles = {}
        nc.gpsimd.dma_start(mask_t[:], mk)
        load_qk(0, eng=nc.sync, sides=(0,))
        load_qk(0, eng=nc.scalar, sides=(1,))
        v_tiles[0] = load_v(0, eng=nc.gpsimd)
        load_qk(1, eng=nc.sync, sides=(0,))
        load_qk(1, eng=nc.scalar, sides=(1,))
        v_tiles[1] = load_v(1, eng=nc.gpsimd)
        load_qk(2, eng=nc.sync, sides=(0,))
        load_qk(2, eng=nc.scalar, sides=(1,))
        v_tiles[2] = load_v(2, eng=nc.gpsimd)
        load_qk(3, eng=nc.sync, sides=(0,))
        load_qk(3, eng=nc.scalar, sides=(1,))
        v_tiles[3] = load_v(3, eng=nc.gpsimd)
        for c0 in range(4, PF):
            load_qk(c0, eng=nc.gpsimd)
            v_tiles[c0] = load_v(c0, eng=nc.gpsimd)

        o_tiles = {}
        pending = []
        for bb in range(NB_TOT + 2):
            if bb < NB_TOT:
                c, bl = bb // NBC, bb % NBC
                if bl == 0:
                    if c + PF < nt:
                        load_qk(c + PF, eng=nc.gpsimd)
                        v_tiles[c + PF] = load_v(c + PF, eng=nc.gpsimd)
                    o_tiles[c] = o_pool.tile([128, GC * 65], F16, name="o_t")
                qk_t = qk_bufs[c % NQK]
                p1 = p1_pool.tile([128, B * 128], F32)
                for j in range(B):
                    g = _slot_group(bl, j)
                    rail, gl = g // 16, g % 16
                    rs = slice(rail * 64, rail * 64 + 64)
                    pc = _p1_col(j)
                    nc.tensor.matmul(
                        p1[:, pc : pc + 128],
                        lhsT=qk_t[rs, QW + gl * 128 : QW + (gl + 1) * 128],
                        rhs=qk_t[rs, gl * 128 : (gl + 1) * 128],
                        start=True,
                        stop=True,
                    )
                w = w_pool.tile([128, B * 128], BF16)
                nc.scalar.activation(
                    w[:], p1[:], mybir.ActivationFunctionType.Exp, scale=SCALE
                )
                wm = wm_pool.tile([128, B * 128], BF16)
                nc.vector.tensor_tensor(
                    wm[:], w[:], mask_t[:], op=mybir.AluOpType.mult
                )
                pending.append((wm, bb))
            if bb >= 2:
                wp, bp = pending.pop(0)
                cp, blp = bp // NBC, bp % NBC
                v_t = v_tiles[cp]
                o_t = o_tiles[cp]
                # slots 0-3 write PSUM bank 0 (cols 0..259), slots 4-7 bank 1
                # (cols 512..771): every matmul-2 output stays inside one
                # bank, and ONE strided CAST copies both halves out
                p2 = p2_pool.tile([128, B * 128], F32)
                for j in range(B):
                    g = _slot_group(blp, j)
                    pc = _p1_col(j)
                    oc = (j // 4) * 512 + (j % 4) * 65
                    nc.tensor.matmul(
                        p2[:, oc : oc + 65],
                        lhsT=wp[:, pc : pc + 128],
                        rhs=v_t[:, g * 65 : (g + 1) * 65],
                        start=True,
                        stop=True,
                    )
                ob = blp * B * 65
                p2v = p2[:].rearrange("p (h c) -> p h c", c=512)
                ovv = o_t[:, ob : ob + 520].rearrange("p (h c) -> p h c", c=260)
                nc.vector.tensor_copy(ovv, p2v[:, :, 0:260])
                if blp % 2 == 1 or cp == nt - 1:
                    # store finished half-chunks to spread write traffic; the
                    # last chunk stores quarter-chunks to shorten the drain
                    oflat = out[cp].rearrange("k p g e -> (k p) (g e)")
                    qw_ = GC * 65 // 4
                    if cp == nt - 1:
                        hsel = slice(blp * qw_, (blp + 1) * qw_)
                    else:
                        hsel = slice(0, 2 * qw_) if blp == 1 else slice(
                            2 * qw_, 4 * qw_
                        )
                    nc.sync.dma_start(oflat[:, hsel], o_t[:, hsel])
                    if blp == NBC - 1:
                        del v_tiles[cp], o_tiles[cp]

    nc.compile()
    return nc


def _prep_qk(qslab: np.ndarray, kslab: np.ndarray, nt: int) -> np.ndarray:
    """Two [nt*CH, 1024] fp32 slabs -> [nt, 2, 128, GC//2, H, 8] bf16.

    Per (chunk, side): one contiguous [128, 2048] block, 4 KiB rows, read
    fully sequentially by one DMA.  Rows 0-63 = d-dim of groups 0-15,
    rows 64-127 = d-dim of groups 16-31."""
    full = np.empty((nt, 2, 2, 64, GC // 2, H, 8), dtype=NPBF16)
    for s, slab in enumerate((qslab, kslab)):
        a = slab.reshape(nt, 2, GC // 2, 8, H, D)   # [c, rail, g, p, h, d]
        full[:, s] = a.transpose(0, 1, 5, 2, 4, 3).astype(NPBF16)
    return full.reshape(nt, 2, 128, GC // 2, H, 8)


def _mask_const() -> np.ndarray:
    """[128, B*128] bf16 0/1 block-diagonal mask.

    Row i = (h', p') of the k-side, column j = (h, p) of the q-side within
    a group; entry is 1 iff p == p' (heads attend only within their own
    position), tiled across the B=8 groups of a batch."""
    m = np.zeros((128, 128), dtype=NPBF16)
    for p in range(8):
        m[(np.arange(H) * 8 + p)[:, None], (np.arange(H) * 8 + p)[None, :]] = 1
    return np.tile(m, (1, B))


def _prep_v(slab: np.ndarray, nt: int) -> np.ndarray:
    """[nt*CH, 1024] fp32 -> [nt, H, 8, GC, 65] bf16 with ones column."""
    a = slab.reshape(nt, GC, 8, H, D)
    full = np.empty((nt, H, 8, GC, 65), dtype=NPBF16)
    full[..., :64] = a.transpose(0, 3, 2, 1, 4).astype(NPBF16)
    full[..., 64] = NPBF16(1.0)
    return full


def kernel(q: np.ndarray, k: np.ndarray, v: np.ndarray) -> np.ndarray:
    bshape = q.shape
    qf = np.ascontiguousarray(np.asarray(q, dtype=np.float32)).reshape(S_TOT, H * D)
    kf = np.ascontiguousarray(np.asarray(k, dtype=np.float32)).reshape(S_TOT, H * D)
    vf = np.ascontiguousarray(np.asarray(v, dtype=np.float32)).reshape(S_TOT, H * D)

    key = (NT, N_CORES)
    if key not in _CACHE:
        _CACHE[key] = _build_program(*key)
    nc = _CACHE[key]

    mk = _mask_const()
    in_maps = []
    for c in range(N_CORES):
        s0, s1 = c * N_PC, (c + 1) * N_PC
        in_maps.append(
            {
                "qk": _prep_qk(qf[s0:s1], kf[s0:s1], NT),
                "mk": mk,
                "vr": _prep_v(vf[s0:s1], NT),
            }
        )

    res = run_bass_kernel_spmd(nc, in_maps, core_ids=list(range(N_CORES)))

    # device column blocks are in pipeline-slot order; map slot -> group
    perm = np.empty(GC, dtype=np.int64)
    for bl in range(NBC):
        for j in range(B):
            perm[_slot_group(bl, j)] = bl * B + j

    out = np.empty((S_TOT, H * D), dtype=np.float32)
    for c in range(N_CORES):
        o = res.results[c]["o"].astype(np.float32)  # [NT, H, 8, GC(slots), 65]
        o = o[:, :, :, perm, :]                     # -> group order
        o = o[..., :64] / o[..., 64:65]             # softmax normalization
        out[c * N_PC : (c + 1) * N_PC] = (
            o.transpose(0, 3, 2, 1, 4).reshape(N_PC, H * D)
        )
    return out.reshape(bshape)
